# revision 22
# baseline (speedup 1.0000x reference)
"""Multi-head attention (B=4, S=1024, D=1024, H=16) on 8 TRN2 NeuronCores.

Sharding: data parallel on batch (4) x tensor parallel on heads (2 groups of
8 heads).  Core c handles batch c//2 and heads (c%2)*8 .. (c%2)*8+7.

Per-core dataflow (everything in "transposed" space so no on-device
transposes are needed):
  QT [512,1024] (d_out on partitions), KT likewise, V [1024,512] natural.
  V2 [k,h,65] = [V*mask | mask]  (65th column -> masked row-sums via matmul)
  scoresT[k,q] = KT_h.T @ QT_h   (K=64 contraction, head pairs row-packed)
  pT = exp(scoresT/8)            (no max subtraction; scores are O(1))
  attnV psum[0:65] = V2_h.T @ pT (rows 0:64 numerator^T, row 64 denominator)
  normalize: fast reciprocal (DVE) + gpsimd partition_broadcast + DVE mult
  Wo: out[q,o] partial = CT.T @ WoT_loc ; host adds the two head-group halves.

Schedule (v2, rebuilt from trace analysis of the previous kernel):
 - The attention phase is Scalar(exp)-bound, projections are PE-bound; the
   previous kernel ran them serially.  Now: Q/K projections run ic-outer
   into all 8 PSUM banks so the PE consumes input chunks as the DMA
   delivers them (kills the 12us startup bubble), and the V projection +
   output projection are woven INTO the attention loop as PE filler while
   the Scalar engine streams exps.
 - exps are emitted per score PAIR [128,1024] (one activation over two
   adjacent PSUM banks) halving per-instruction overhead on Scalar.
 - the softmax normalization broadcast is gpsimd.partition_broadcast
   instead of a PE ones-matmul (PE -3.5us, and DVE no longer copies the
   broadcast result out of PSUM).
 - Scalar issues NO DMAs and only runs exps (plus 4 early projection
   drains); input DMA issue is round-robined over Sync/GpSimd/Vector,
   output DMA on Sync.
 - all matmul operands bfloat16 (fp8 DoubleRow was measured offline at
   rel-err 0.022-0.032 > the 2e-2 gate, so everything stays bf16).
End-to-end rel err ~5e-3 (gate 2e-2).
"""
import sys

if '/opt/trn_rl_repo' not in sys.path:
    sys.path.insert(0, '/opt/trn_rl_repo')

import numpy as np

P = 128
B, S, D = 4, 1024, 1024
DL = 512          # local d_out (8 heads x 64)
H = 8             # local heads
E = 64            # head dim
IC = D // P       # 8 contraction chunks for projections
KC = S // P       # 8 key-position chunks
T4 = DL // P      # 4 tiles holding QT/KT/CT rows
NQ = 512          # matmul moving free dim
NC2 = IC // 2     # chunk pairs
N_CORES = 8

_prog_cache = {}

MM_DTYPE = 'bf16'


def build_program(mm_dtype=None, nkc=KC):
    import concourse.tile as tile
    from concourse import bacc, mybir

    F32 = mybir.dt.float32
    F16 = (mybir.dt.bfloat16 if (mm_dtype or MM_DTYPE) == 'bf16'
           else mybir.dt.float16)
    OUT16 = mybir.dt.float16
    EXP = mybir.ActivationFunctionType.Exp
    COPY = mybir.ActivationFunctionType.Copy
    MULT = mybir.AluOpType.mult

    nc = bacc.Bacc("TRN2", target_bir_lowering=False, debug=False,
                   enable_asserts=False, num_devices=N_CORES)

    xtq = nc.dram_tensor("xtq", (D, S), F16, kind="ExternalInput").ap()
    xtk = nc.dram_tensor("xtk", (D, S), F16, kind="ExternalInput").ap()
    xtv = nc.dram_tensor("xtv", (D, S), F16, kind="ExternalInput").ap()
    wq = nc.dram_tensor("wq", (D, DL), F16, kind="ExternalInput").ap()
    wk = nc.dram_tensor("wk", (D, DL), F16, kind="ExternalInput").ap()
    wv = nc.dram_tensor("wv", (D, DL), F16, kind="ExternalInput").ap()
    wo = nc.dram_tensor("wo", (DL, D), F16, kind="ExternalInput").ap()
    maskd = nc.dram_tensor("maskd", (P, KC), F32, kind="ExternalInput").ap()
    out = nc.dram_tensor("out", (S, D), OUT16, kind="ExternalOutput").ap()

    # chunk-pair views: (c p two s) so one dma_start loads 2 contraction
    # chunks into a [P, 2, *] SBUF tile
    xtq_c = xtq.rearrange("(c two p) s -> c p two s", two=2, p=P)
    xtk_c = xtk.rearrange("(c two p) s -> c p two s", two=2, p=P)
    xtv_c = xtv.rearrange("(c two p) s -> c p two s", two=2, p=P)
    wq_c = wq.rearrange("(c two p) o -> c p two o", two=2, p=P)
    wk_c = wk.rearrange("(c two p) o -> c p two o", two=2, p=P)
    wv_c = wv.rearrange("(c two p) o -> c p two o", two=2, p=P)
    wo_c = wo.rearrange("(t p) o -> p t o", p=P)

    npair = nkc // 2
    nsing = nkc % 2
    ntile = npair + nsing   # score psum tiles per (h, qc) iteration

    from concourse import library_config

    with tile.TileContext(nc) as tc:
        with tc.tile_pool(name="xp", bufs=12) as x_pool, \
             tc.tile_pool(name="wp", bufs=12) as w_pool, \
             tc.tile_pool(name="wop", bufs=1) as wo_pool, \
             tc.tile_pool(name="qk", bufs=8) as qk_pool, \
             tc.tile_pool(name="v2p", bufs=8) as v2_pool, \
             tc.tile_pool(name="pp", bufs=30) as p_pool, \
             tc.tile_pool(name="ctp", bufs=4) as ct_pool, \
             tc.tile_pool(name="sm", bufs=2) as small, \
             tc.tile_pool(name="rbp", bufs=3) as rb_pool, \
             tc.tile_pool(name="ob", bufs=4) as out_pool, \
             tc.tile_pool(name="psS", bufs=2, space="PSUM") as psS, \
             tc.tile_pool(name="psX", bufs=4, space="PSUM") as psX:

            # ---- constants / small inputs ----
            mask_sb = small.tile([P, KC], F32, tag="mask")
            nc.sync.dma_start(mask_sb[:], maskd[:])

            dma_engines = [nc.sync, nc.gpsimd, nc.scalar]
            dma_rr = [0]

            def dma_load(dst, src, eng=None):
                if eng is None:
                    eng = dma_engines[dma_rr[0] % len(dma_engines)]
                    dma_rr[0] += 1
                eng.dma_start(dst, src)

            # ---- input loads: chunk pairs, consumption order ----
            # split=True loads the first pair as two single-chunk DMAs so
            # the very first projection matmul starts ~2us earlier
            def load_pairs(w_dram, x_dram, wname, xname, split=False):
                w_tiles, x_tiles = [], []
                for c in range(NC2):
                    wt = w_pool.tile([P, 2, DL], F16, tag="wp",
                                     name=f"{wname}{c}")
                    xt = x_pool.tile([P, 2, S], F16, tag="xt",
                                     name=f"{xname}{c}")
                    if split and c == 0:
                        for i in range(2):
                            dma_load(wt[:, i, :], w_dram[0, :, i, :])
                            dma_load(xt[:, i, :], x_dram[0, :, i, :])
                    else:
                        dma_load(wt[:], w_dram[c])
                        dma_load(xt[:], x_dram[c])
                    w_tiles.append(wt)
                    x_tiles.append(xt)
                return w_tiles, x_tiles

            # q/k chunk pairs interleaved so phase-1 (Q+K t0..t2) can
            # consume both streams as they arrive
            wq_sb, xq_sb = load_pairs(wq_c, xtq_c, "wq", "xq")
            wk_sb, xk_sb = load_pairs(wk_c, xtk_c, "wk", "xk")
            wv_sb, xv_sb = load_pairs(wv_c, xtv_c, "wv", "xv")
            wo_sb = wo_pool.tile([P, T4, D], F16, tag="wo")
            dma_load(wo_sb[:], wo_c[:])
            # ucode load for partition_broadcast; AFTER the input dma issues
            # (the ucode DMA stalls gpsimd ~11us, and the first broadcast
            # isn't needed until much later)
            nc.gpsimd.load_library(library_config.attn)

            qt = [qk_pool.tile([P, S], F16, tag="qk", name=f"qt{i}")
                  for i in range(T4)]
            kt = [qk_pool.tile([P, S], F16, tag="qk", name=f"kt{i}")
                  for i in range(T4)]

            # ---- Q / K projections, two waves each ----
            # wave 1 (t0/t1 -> psS halves) + wave 2 (t2/t3 -> psX).  Q
            # waves run back to back (DMA-paced at first); K wave 1 runs
            # as soon as the K chunks land, and its drain releases psS to
            # the score stream ~14us earlier than a monolithic K
            # projection.  K wave 2 is handed to the filler FIFO.
            def proj_wave1(w_sb, x_sb, dest):
                pss = [psS.tile([P, 2 * NQ], F32, tag="psS", name="prjS0"),
                       psS.tile([P, 2 * NQ], F32, tag="psS", name="prjS1")]
                accs = [pss[0][:, 0:NQ], pss[0][:, NQ:2 * NQ],
                        pss[1][:, 0:NQ], pss[1][:, NQ:2 * NQ]]
                for ic in range(IC):
                    c, i = divmod(ic, 2)
                    for g in range(4):
                        t, sc = divmod(g, 2)
                        nc.tensor.matmul(
                            accs[g],
                            w_sb[c][:, i, t * P:(t + 1) * P],
                            x_sb[c][:, i, sc * NQ:(sc + 1) * NQ],
                            start=(ic == 0), stop=(ic == IC - 1))
                nc.scalar.activation(dest[0][:, :], pss[0][:, :], COPY)
                nc.scalar.activation(dest[1][:, :], pss[1][:, :], COPY)

            def proj_w2_group(w_sb, x_sb, dest, g):
                t, sc = divmod(g, 2)
                ps = psX.tile([P, NQ], F32, tag="psX", name="prjX")
                for ic in range(IC):
                    c, i = divmod(ic, 2)
                    nc.tensor.matmul(
                        ps[:],
                        w_sb[c][:, i, (t + 2) * P:(t + 3) * P],
                        x_sb[c][:, i, sc * NQ:(sc + 1) * NQ],
                        start=(ic == 0), stop=(ic == IC - 1))
                nc.vector.tensor_copy(
                    dest[t + 2][:, sc * NQ:(sc + 1) * NQ], ps[:])

            proj_wave1(wq_sb, xq_sb, qt)
            for g in range(4):
                proj_w2_group(wq_sb, xq_sb, qt, g)
            proj_wave1(wk_sb, xk_sb, kt)

            # ---- V projection groups (emitted as PE filler below) ----
            v2 = []

            def emit_v_group(skc):
                ps = psX.tile([P, NQ], F32, tag="psX")
                for ic in range(IC):
                    c, i = divmod(ic, 2)
                    nc.tensor.matmul(
                        ps[:],
                        xv_sb[c][:, i, skc * P:(skc + 1) * P],
                        wv_sb[c][:, i, :],
                        start=(ic == 0), stop=(ic == IC - 1))
                v2t = v2_pool.tile([P, H, E + 1], F16, tag="v2")
                nc.vector.tensor_scalar_mul(
                    v2t[:, :, 0:E],
                    ps[:].rearrange("p (h e) -> p h e", h=H),
                    mask_sb[:, skc:skc + 1])
                nc.vector.tensor_copy(
                    v2t[:, :, E:E + 1],
                    mask_sb[:, skc:skc + 1, None].to_broadcast((P, H, 1)))
                v2.append(v2t)

            # ---- output projection groups (PE filler + tail) ----
            ct = [ct_pool.tile([P, S], F16, tag="ct", name=f"ct{i}")
                  for i in range(T4)]

            out_rr = [0]
            out_engs = [nc.sync, nc.gpsimd]

            def emit_wo_part(qc8, oc, t1):
                ps = psX.tile([P, NQ], F32, tag="psX", name="wops")
                for t in range(t1):
                    nc.tensor.matmul(
                        ps[:],
                        ct[t][:, qc8 * P:(qc8 + 1) * P],
                        wo_sb[:, t, oc * NQ:(oc + 1) * NQ],
                        start=(t == 0), stop=(t1 == T4 and t == T4 - 1))
                return ps

            def emit_wo_finish(qc8, oc, ps, t0=T4):
                for t in range(t0, T4):
                    nc.tensor.matmul(
                        ps[:],
                        ct[t][:, qc8 * P:(qc8 + 1) * P],
                        wo_sb[:, t, oc * NQ:(oc + 1) * NQ],
                        start=False, stop=(t == T4 - 1))
                osb = out_pool.tile([P, NQ], OUT16, tag="osb")
                nc.vector.tensor_copy(osb[:], ps[:])
                oeng = out_engs[out_rr[0] % len(out_engs)]
                out_rr[0] += 1
                dma_load(
                    out[qc8 * P:(qc8 + 1) * P, oc * NQ:(oc + 1) * NQ],
                    osb[:], eng=oeng)

            def emit_wo_group(qc8, oc):
                emit_wo_finish(qc8, oc, emit_wo_part(qc8, oc, T4))

            # ---- attention: uniform score-chunk stream ----
            # All 16*nkc score chunks form one global stream, paired into
            # [P,1024] psum tiles (one exp each; 16*nkc is always even, so
            # every exp is a full pair — pairs may straddle iteration
            # boundaries).  PE filler work (phase-2 projections, V
            # projection, attnV+normalize, output projection) is popped
            # from a FIFO by a leaky bucket so score tiles reach the
            # Scalar engine at a steady cadence instead of in bursts.
            iters = [(h, qc) for qc in range(2) for h in range(H)]
            chunks = [(it, kc) for it in range(len(iters))
                      for kc in range(nkc)]
            n_tiles = (len(chunks) + 1) // 2
            preg = {}          # (it, kc) -> (p_tile, half)

            def emit_attnv_norm(h, qc, it):
                t, half = h // 2, h % 2
                pb = half * E
                pso = psX.tile([P, NQ], F32, tag="psX", name="avps")
                for kc in range(nkc):
                    pt, u = preg[(it, kc)]
                    nc.tensor.matmul(
                        pso[0:E + 1, :],
                        v2[kc][:, h, :],
                        pt[:, u * NQ:(u + 1) * NQ],
                        start=(kc == 0), stop=(kc == nkc - 1))
                # normalize off the PE: recip (DVE) -> partition_broadcast
                # (gpsimd) -> multiply (DVE)
                tmp = small.tile([1, 2 * NQ], F32, tag="ntmp")
                d_ = tmp[0:1, 0:NQ]
                r_ = tmp[0:1, NQ:2 * NQ]
                nc.vector.tensor_copy(d_, pso[E:E + 1, :])
                nc.vector.reciprocal_approx_fast(r_, d_)
                rb = rb_pool.tile([E, NQ], F32, tag="rb")
                nc.gpsimd.partition_broadcast(rb[:], r_)
                nc.vector.tensor_tensor(
                    ct[t][pb:pb + E, qc * NQ:(qc + 1) * NQ],
                    pso[0:E, :], rb[:], op=MULT)

            wo_groups = [(qc8, oc) for qc8 in range(KC) for oc in range(2)]
            fillers = []       # (min_tile, est_ns, fn, kind) emission order
            for g in range(4):
                fillers.append(
                    (0, 1800,
                     (lambda g=g: proj_w2_group(wk_sb, xk_sb, kt, g)), 'k2'))
            for s in range(nkc):
                fillers.append((0, 1800, (lambda s=s: emit_v_group(s)), 'v'))
            for i, (h, qc) in enumerate(iters):
                lt = (i * nkc + nkc - 1) // 2
                fillers.append(
                    (lt + 2, 220 * nkc + 60,
                     (lambda h=h, qc=qc, i=i: emit_attnv_norm(h, qc, i)),
                     'av'))
                if (h, qc) == (H - 1, 0):
                    for g in range(8):
                        fillers.append(
                            (0, 950,
                             (lambda g=g: emit_wo_group(*wo_groups[g])),
                             'wo'))

            in_stream = sum(e for mt, e, _, _ in fillers if mt < n_tiles)
            rate = in_stream / n_tiles
            budget = 2000.0
            fi = 0
            for tk in range(n_tiles):
                pair = chunks[2 * tk:2 * tk + 2]
                pss = psS.tile([P, 2 * NQ], F32, tag="psS", name="sps")
                pt = p_pool.tile([P, 2 * NQ], F16, tag="pt", name="pt")
                for u, (it, kc) in enumerate(pair):
                    h, qc = iters[it]
                    t, half = h // 2, h % 2
                    pb = half * E
                    nc.tensor.matmul(
                        pss[:, u * NQ:(u + 1) * NQ],
                        kt[t][pb:pb + E, kc * P:(kc + 1) * P],
                        qt[t][pb:pb + E, qc * NQ:(qc + 1) * NQ],
                        start=True, stop=True,
                        tile_position=(pb, 0))
                    preg[(it, kc)] = (pt, u)
                w = len(pair)
                nc.scalar.activation(pt[:, 0:w * NQ], pss[:, 0:w * NQ],
                                     EXP, scale=0.125)
                budget += rate
                while (fi < len(fillers) and fillers[fi][0] <= tk
                       and fillers[fi][1] <= budget):
                    budget -= fillers[fi][1]
                    fillers[fi][2]()
                    fi += 1
            # endgame: leftover in-stream fillers except the last attnV
            rem = list(fillers[fi:])
            last_av = max(k for k, f in enumerate(rem) if f[3] == 'av')
            for _, _, fn, _ in rem[:last_av]:
                fn()
            # pre-accumulate the t0..t2 partials of three qc=1 output
            # groups while the PE would otherwise wait for the last exp /
            # normalize chain, then run the final attnV and finish up
            parts = [(qc8, oc, emit_wo_part(qc8, oc, T4 - 1))
                     for qc8, oc in wo_groups[8:11]]
            rem[last_av][2]()
            for _, _, fn, _ in rem[last_av + 1:]:
                fn()
            # four more partials in the now-free psS halves (two groups
            # per tile so the 2-buf pool is not wrapped): this work covers
            # the final recip/broadcast/multiply latency
            pstl = [psS.tile([P, 2 * NQ], F32, tag="psS", name="wops2a"),
                    psS.tile([P, 2 * NQ], F32, tag="psS", name="wops2b")]
            for j, (qc8, oc) in enumerate(wo_groups[11:15]):
                acc = pstl[j // 2][:, (j % 2) * NQ:(j % 2 + 1) * NQ]
                for t in range(T4 - 1):
                    nc.tensor.matmul(
                        acc,
                        ct[t][:, qc8 * P:(qc8 + 1) * P],
                        wo_sb[:, t, oc * NQ:(oc + 1) * NQ],
                        start=(t == 0), stop=False)
                parts.append((qc8, oc, acc))
            out_engs.append(nc.scalar)
            for qc8, oc, ps in parts:
                emit_wo_finish(qc8, oc, ps, t0=T4 - 1)
            for qc8, oc in wo_groups[15:]:
                emit_wo_group(qc8, oc)

    nc.compile()
    return nc


def make_in_maps(queries, keys, values, valid_lens, W_q, W_k, W_v, W_o):
    queries = np.asarray(queries, dtype=np.float32)
    keys = np.asarray(keys, dtype=np.float32)
    values = np.asarray(values, dtype=np.float32)
    valid_lens = np.asarray(valid_lens)
    W_q = np.asarray(W_q, dtype=np.float32)
    W_k = np.asarray(W_k, dtype=np.float32)
    W_v = np.asarray(W_v, dtype=np.float32)
    W_o = np.asarray(W_o, dtype=np.float32)

    if MM_DTYPE == 'bf16':
        import ml_dtypes
        f16 = np.dtype(ml_dtypes.bfloat16)
    else:
        f16 = np.float16
    xtq = [np.ascontiguousarray(queries[b].T.astype(f16)) for b in range(B)]
    xtk = [np.ascontiguousarray(keys[b].T.astype(f16)) for b in range(B)]
    xtv = [np.ascontiguousarray(values[b].T.astype(f16)) for b in range(B)]
    wqt = [np.ascontiguousarray(W_q[hg * DL:(hg + 1) * DL, :].T.astype(f16))
           for hg in range(2)]
    wkt = [np.ascontiguousarray(W_k[hg * DL:(hg + 1) * DL, :].T.astype(f16))
           for hg in range(2)]
    wvt = [np.ascontiguousarray(W_v[hg * DL:(hg + 1) * DL, :].T.astype(f16))
           for hg in range(2)]
    wot = [np.ascontiguousarray(W_o[:, hg * DL:(hg + 1) * DL].T.astype(f16))
           for hg in range(2)]

    in_maps = []
    for c in range(N_CORES):
        b, hg = c // 2, c % 2
        L = int(valid_lens[b])
        k_idx = np.arange(S).reshape(KC, P).T  # [P, KC]
        maskd = (k_idx < L).astype(np.float32)
        in_maps.append({
            "xtq": xtq[b], "xtk": xtk[b], "xtv": xtv[b],
            "wq": wqt[hg], "wk": wkt[hg], "wv": wvt[hg], "wo": wot[hg],
            "maskd": np.ascontiguousarray(maskd),
        })
    return in_maps


def gather(results):
    out = np.empty((B, S, D), dtype=np.float32)
    for b in range(B):
        out[b] = (results[2 * b]["out"].astype(np.float32)
                  + results[2 * b + 1]["out"].astype(np.float32))
    return out


def kernel(queries, keys, values, valid_lens, W_q, W_k, W_v, W_o):
    from concourse.bass_utils import run_bass_kernel_spmd

    # key chunks >= ceil(max(valid_lens)/128) are fully masked on every
    # core and contribute exactly zero to numerator and denominator:
    # skip them in the attention loops.
    nkc = max(1, min(KC, -(-int(np.max(np.asarray(valid_lens))) // P)))
    if nkc not in _prog_cache:
        _prog_cache[nkc] = build_program(nkc=nkc)
    nc = _prog_cache[nkc]

    in_maps = make_in_maps(queries, keys, values, valid_lens,
                           W_q, W_k, W_v, W_o)
    res = run_bass_kernel_spmd(nc, in_maps, core_ids=list(range(N_CORES)))
    return gather(res.results)


# revision 23
# speedup vs baseline: 1.0687x; 1.0687x over previous
"""Multi-head attention (B=4, S=1024, D=1024, H=16) on 8 TRN2 NeuronCores.

Sharding: data parallel on batch (4) x tensor parallel on heads (2 groups of
8 heads).  Core c handles batch c//2 and heads (c%2)*8 .. (c%2)*8+7.

Per-core dataflow (everything in "transposed" space so no on-device
transposes are needed):
  QT [512,1024] (d_out on partitions), KT likewise, V [1024,512] natural.
  V2 [k,h,65] = [V*mask | mask]  (65th column -> masked row-sums via matmul)
  scoresT[k,q] = KT_h.T @ QT_h   (K=64 contraction, head pairs row-packed)
  pT = exp(scoresT/8)            (no max subtraction; scores are O(1))
  attnV psum[0:65] = V2_h.T @ pT (rows 0:64 numerator^T, row 64 denominator)
  normalize: fast reciprocal (DVE) + gpsimd partition_broadcast + DVE mult
  Wo: out[q,o] partial = CT.T @ WoT_loc ; host adds the two head-group halves.

Schedule (v2, rebuilt from trace analysis of the previous kernel):
 - The attention phase is Scalar(exp)-bound, projections are PE-bound; the
   previous kernel ran them serially.  Now: Q/K projections run ic-outer
   into all 8 PSUM banks so the PE consumes input chunks as the DMA
   delivers them (kills the 12us startup bubble), and the V projection +
   output projection are woven INTO the attention loop as PE filler while
   the Scalar engine streams exps.
 - exps are emitted per score PAIR [128,1024] (one activation over two
   adjacent PSUM banks) halving per-instruction overhead on Scalar.
 - the softmax normalization broadcast is gpsimd.partition_broadcast
   instead of a PE ones-matmul (PE -3.5us, and DVE no longer copies the
   broadcast result out of PSUM).
 - Scalar issues NO DMAs and only runs exps (plus 4 early projection
   drains); input DMA issue is round-robined over Sync/GpSimd/Vector,
   output DMA on Sync.
 - all matmul operands bfloat16 (fp8 DoubleRow was measured offline at
   rel-err 0.022-0.032 > the 2e-2 gate, so everything stays bf16).
End-to-end rel err ~5e-3 (gate 2e-2).
"""
import sys

if '/opt/trn_rl_repo' not in sys.path:
    sys.path.insert(0, '/opt/trn_rl_repo')

import numpy as np

P = 128
B, S, D = 4, 1024, 1024
DL = 512          # local d_out (8 heads x 64)
H = 8             # local heads
E = 64            # head dim
IC = D // P       # 8 contraction chunks for projections
KC = S // P       # 8 key-position chunks
T4 = DL // P      # 4 tiles holding QT/KT/CT rows
NQ = 512          # matmul moving free dim
NC2 = IC // 2     # chunk pairs
N_CORES = 8

_prog_cache = {}

MM_DTYPE = 'bf16'


def build_program(mm_dtype=None, nkc=KC):
    import concourse.tile as tile
    from concourse import bacc, mybir

    F32 = mybir.dt.float32
    F16 = (mybir.dt.bfloat16 if (mm_dtype or MM_DTYPE) == 'bf16'
           else mybir.dt.float16)
    OUT16 = mybir.dt.float16
    EXP = mybir.ActivationFunctionType.Exp
    COPY = mybir.ActivationFunctionType.Copy
    MULT = mybir.AluOpType.mult

    nc = bacc.Bacc("TRN2", target_bir_lowering=False, debug=False,
                   enable_asserts=False, num_devices=N_CORES)

    xtq = nc.dram_tensor("xtq", (D, S), F16, kind="ExternalInput").ap()
    xtk = nc.dram_tensor("xtk", (D, S), F16, kind="ExternalInput").ap()
    xtv = nc.dram_tensor("xtv", (D, S), F16, kind="ExternalInput").ap()
    wq = nc.dram_tensor("wq", (D, DL), F16, kind="ExternalInput").ap()
    wk = nc.dram_tensor("wk", (D, DL), F16, kind="ExternalInput").ap()
    wv = nc.dram_tensor("wv", (D, DL), F16, kind="ExternalInput").ap()
    wo = nc.dram_tensor("wo", (DL, D), F16, kind="ExternalInput").ap()
    maskd = nc.dram_tensor("maskd", (P, KC), F32, kind="ExternalInput").ap()
    out = nc.dram_tensor("out", (S, D), OUT16, kind="ExternalOutput").ap()

    # chunk-pair views: (c p two s) so one dma_start loads 2 contraction
    # chunks into a [P, 2, *] SBUF tile
    xtq_c = xtq.rearrange("(c two p) s -> c p two s", two=2, p=P)
    xtk_c = xtk.rearrange("(c two p) s -> c p two s", two=2, p=P)
    xtv_c = xtv.rearrange("(c two p) s -> c p two s", two=2, p=P)
    wq_c = wq.rearrange("(c two p) o -> c p two o", two=2, p=P)
    wk_c = wk.rearrange("(c two p) o -> c p two o", two=2, p=P)
    wv_c = wv.rearrange("(c two p) o -> c p two o", two=2, p=P)
    wo_c = wo.rearrange("(t p) o -> p t o", p=P)

    npair = nkc // 2
    nsing = nkc % 2
    ntile = npair + nsing   # score psum tiles per (h, qc) iteration

    from concourse import library_config

    with tile.TileContext(nc) as tc:
        with tc.tile_pool(name="xp", bufs=12) as x_pool, \
             tc.tile_pool(name="wp", bufs=12) as w_pool, \
             tc.tile_pool(name="wop", bufs=1) as wo_pool, \
             tc.tile_pool(name="qk", bufs=8) as qk_pool, \
             tc.tile_pool(name="v2p", bufs=8) as v2_pool, \
             tc.tile_pool(name="pp", bufs=30) as p_pool, \
             tc.tile_pool(name="ctp", bufs=4) as ct_pool, \
             tc.tile_pool(name="sm", bufs=2) as small, \
             tc.tile_pool(name="rbp", bufs=3) as rb_pool, \
             tc.tile_pool(name="ob", bufs=4) as out_pool, \
             tc.tile_pool(name="psS", bufs=2, space="PSUM") as psS, \
             tc.tile_pool(name="psX", bufs=4, space="PSUM") as psX:

            # ---- constants / small inputs ----
            mask_sb = small.tile([P, KC], F32, tag="mask")
            nc.sync.dma_start(mask_sb[:], maskd[:])

            dma_engines = [nc.sync, nc.gpsimd, nc.scalar]
            dma_rr = [0]

            def dma_load(dst, src, eng=None):
                if eng is None:
                    eng = dma_engines[dma_rr[0] % len(dma_engines)]
                    dma_rr[0] += 1
                eng.dma_start(dst, src)

            # the gpsimd DMA queue takes ~15-20us to deliver its first
            # packet, so the early-needed q/k chunks ride sync+scalar only
            qk_engines = [nc.sync, nc.scalar]
            qk_rr = [0]

            def dma_load_qk(dst, src):
                eng = qk_engines[qk_rr[0] % 2]
                qk_rr[0] += 1
                eng.dma_start(dst, src)

            # ---- input loads: chunk pairs, consumption order ----
            # split=True loads the first pair as two single-chunk DMAs so
            # the very first projection matmul starts ~2us earlier
            def load_pairs(w_dram, x_dram, wname, xname, early=False):
                ld = dma_load_qk if early else dma_load
                w_tiles, x_tiles = [], []
                for c in range(NC2):
                    wt = w_pool.tile([P, 2, DL], F16, tag="wp",
                                     name=f"{wname}{c}")
                    xt = x_pool.tile([P, 2, S], F16, tag="xt",
                                     name=f"{xname}{c}")
                    ld(wt[:], w_dram[c])
                    ld(xt[:], x_dram[c])
                    w_tiles.append(wt)
                    x_tiles.append(xt)
                return w_tiles, x_tiles

            # q/k chunk pairs interleaved so phase-1 (Q+K t0..t2) can
            # consume both streams as they arrive
            wq_sb, xq_sb = load_pairs(wq_c, xtq_c, "wq", "xq", early=True)
            wk_sb, xk_sb = load_pairs(wk_c, xtk_c, "wk", "xk", early=True)
            wv_sb, xv_sb = load_pairs(wv_c, xtv_c, "wv", "xv")
            wo_sb = wo_pool.tile([P, T4, D], F16, tag="wo")
            dma_load(wo_sb[:], wo_c[:])
            # ucode load for partition_broadcast; AFTER the input dma issues
            # (the ucode DMA stalls gpsimd ~11us, and the first broadcast
            # isn't needed until much later)
            nc.gpsimd.load_library(library_config.attn)

            qt = [qk_pool.tile([P, S], F16, tag="qk", name=f"qt{i}")
                  for i in range(T4)]
            kt = [qk_pool.tile([P, S], F16, tag="qk", name=f"kt{i}")
                  for i in range(T4)]

            # ---- Q / K projections, two waves each ----
            # wave 1 (t0/t1 -> psS halves) + wave 2 (t2/t3 -> psX).  Q
            # waves run back to back (DMA-paced at first); K wave 1 runs
            # as soon as the K chunks land, and its drain releases psS to
            # the score stream ~14us earlier than a monolithic K
            # projection.  K wave 2 is handed to the filler FIFO.
            def proj_wave1(w_sb, x_sb, dest):
                pss = [psS.tile([P, 2 * NQ], F32, tag="psS", name="prjS0"),
                       psS.tile([P, 2 * NQ], F32, tag="psS", name="prjS1")]
                accs = [pss[0][:, 0:NQ], pss[0][:, NQ:2 * NQ],
                        pss[1][:, 0:NQ], pss[1][:, NQ:2 * NQ]]
                for ic in range(IC):
                    c, i = divmod(ic, 2)
                    for g in range(4):
                        t, sc = divmod(g, 2)
                        nc.tensor.matmul(
                            accs[g],
                            w_sb[c][:, i, t * P:(t + 1) * P],
                            x_sb[c][:, i, sc * NQ:(sc + 1) * NQ],
                            start=(ic == 0), stop=(ic == IC - 1))
                nc.scalar.activation(dest[0][:, :], pss[0][:, :], COPY)
                nc.scalar.activation(dest[1][:, :], pss[1][:, :], COPY)

            def proj_w2_group(w_sb, x_sb, dest, g):
                t, sc = divmod(g, 2)
                ps = psX.tile([P, NQ], F32, tag="psX", name="prjX")
                for ic in range(IC):
                    c, i = divmod(ic, 2)
                    nc.tensor.matmul(
                        ps[:],
                        w_sb[c][:, i, (t + 2) * P:(t + 3) * P],
                        x_sb[c][:, i, sc * NQ:(sc + 1) * NQ],
                        start=(ic == 0), stop=(ic == IC - 1))
                nc.vector.tensor_copy(
                    dest[t + 2][:, sc * NQ:(sc + 1) * NQ], ps[:])

            proj_wave1(wq_sb, xq_sb, qt)
            for g in range(4):
                proj_w2_group(wq_sb, xq_sb, qt, g)
            proj_wave1(wk_sb, xk_sb, kt)

            # ---- V projection groups (emitted as PE filler below) ----
            v2 = []

            def emit_v_group(skc):
                ps = psX.tile([P, NQ], F32, tag="psX")
                for ic in range(IC):
                    c, i = divmod(ic, 2)
                    nc.tensor.matmul(
                        ps[:],
                        xv_sb[c][:, i, skc * P:(skc + 1) * P],
                        wv_sb[c][:, i, :],
                        start=(ic == 0), stop=(ic == IC - 1))
                v2t = v2_pool.tile([P, H, E + 1], F16, tag="v2")
                nc.vector.tensor_scalar_mul(
                    v2t[:, :, 0:E],
                    ps[:].rearrange("p (h e) -> p h e", h=H),
                    mask_sb[:, skc:skc + 1])
                nc.vector.tensor_copy(
                    v2t[:, :, E:E + 1],
                    mask_sb[:, skc:skc + 1, None].to_broadcast((P, H, 1)))
                v2.append(v2t)

            # ---- output projection groups (PE filler + tail) ----
            ct = [ct_pool.tile([P, S], F16, tag="ct", name=f"ct{i}")
                  for i in range(T4)]

            out_rr = [0]
            out_engs = [nc.sync, nc.gpsimd]

            def emit_wo_part(qc8, oc, t1):
                ps = psX.tile([P, NQ], F32, tag="psX", name="wops")
                for t in range(t1):
                    nc.tensor.matmul(
                        ps[:],
                        ct[t][:, qc8 * P:(qc8 + 1) * P],
                        wo_sb[:, t, oc * NQ:(oc + 1) * NQ],
                        start=(t == 0), stop=(t1 == T4 and t == T4 - 1))
                return ps

            def emit_wo_finish(qc8, oc, ps, t0=T4):
                for t in range(t0, T4):
                    nc.tensor.matmul(
                        ps[:],
                        ct[t][:, qc8 * P:(qc8 + 1) * P],
                        wo_sb[:, t, oc * NQ:(oc + 1) * NQ],
                        start=False, stop=(t == T4 - 1))
                osb = out_pool.tile([P, NQ], OUT16, tag="osb")
                nc.vector.tensor_copy(osb[:], ps[:])
                oeng = out_engs[out_rr[0] % len(out_engs)]
                out_rr[0] += 1
                dma_load(
                    out[qc8 * P:(qc8 + 1) * P, oc * NQ:(oc + 1) * NQ],
                    osb[:], eng=oeng)

            def emit_wo_group(qc8, oc):
                emit_wo_finish(qc8, oc, emit_wo_part(qc8, oc, T4))

            # ---- attention: uniform score-chunk stream ----
            # All 16*nkc score chunks form one global stream, paired into
            # [P,1024] psum tiles (one exp each; 16*nkc is always even, so
            # every exp is a full pair — pairs may straddle iteration
            # boundaries).  PE filler work (phase-2 projections, V
            # projection, attnV+normalize, output projection) is popped
            # from a FIFO by a leaky bucket so score tiles reach the
            # Scalar engine at a steady cadence instead of in bursts.
            iters = [(h, qc) for qc in range(2) for h in range(H)]
            chunks = [(it, kc) for it in range(len(iters))
                      for kc in range(nkc)]
            n_tiles = (len(chunks) + 1) // 2
            preg = {}          # (it, kc) -> (p_tile, half)

            def emit_attnv_norm(h, qc, it):
                t, half = h // 2, h % 2
                pb = half * E
                pso = psX.tile([P, NQ], F32, tag="psX", name="avps")
                for kc in range(nkc):
                    pt, u = preg[(it, kc)]
                    nc.tensor.matmul(
                        pso[0:E + 1, :],
                        v2[kc][:, h, :],
                        pt[:, u * NQ:(u + 1) * NQ],
                        start=(kc == 0), stop=(kc == nkc - 1))
                # normalize off the PE: recip (DVE) -> partition_broadcast
                # (gpsimd) -> multiply (DVE)
                tmp = small.tile([1, 2 * NQ], F32, tag="ntmp")
                d_ = tmp[0:1, 0:NQ]
                r_ = tmp[0:1, NQ:2 * NQ]
                nc.vector.tensor_copy(d_, pso[E:E + 1, :])
                nc.vector.reciprocal_approx_fast(r_, d_)
                rb = rb_pool.tile([E, NQ], F32, tag="rb")
                nc.gpsimd.partition_broadcast(rb[:], r_)
                nc.vector.tensor_tensor(
                    ct[t][pb:pb + E, qc * NQ:(qc + 1) * NQ],
                    pso[0:E, :], rb[:], op=MULT)

            wo_groups = [(qc8, oc) for qc8 in range(KC) for oc in range(2)]
            fillers = []       # (min_tile, est_ns, fn, kind) emission order
            for g in range(4):
                fillers.append(
                    (0, 1800,
                     (lambda g=g: proj_w2_group(wk_sb, xk_sb, kt, g)), 'k2'))
            for s in range(nkc):
                fillers.append((0, 1800, (lambda s=s: emit_v_group(s)), 'v'))
            for i, (h, qc) in enumerate(iters):
                lt = (i * nkc + nkc - 1) // 2
                fillers.append(
                    (lt + 2, 220 * nkc + 60,
                     (lambda h=h, qc=qc, i=i: emit_attnv_norm(h, qc, i)),
                     'av'))
                if (h, qc) == (H - 1, 0):
                    for g in range(8):
                        fillers.append(
                            (0, 950,
                             (lambda g=g: emit_wo_group(*wo_groups[g])),
                             'wo'))

            in_stream = sum(e for mt, e, _, _ in fillers if mt < n_tiles)
            rate = in_stream / n_tiles
            budget = 2000.0
            fi = 0
            for tk in range(n_tiles):
                pair = chunks[2 * tk:2 * tk + 2]
                pss = psS.tile([P, 2 * NQ], F32, tag="psS", name="sps")
                pt = p_pool.tile([P, 2 * NQ], F16, tag="pt", name="pt")
                for u, (it, kc) in enumerate(pair):
                    h, qc = iters[it]
                    t, half = h // 2, h % 2
                    pb = half * E
                    nc.tensor.matmul(
                        pss[:, u * NQ:(u + 1) * NQ],
                        kt[t][pb:pb + E, kc * P:(kc + 1) * P],
                        qt[t][pb:pb + E, qc * NQ:(qc + 1) * NQ],
                        start=True, stop=True,
                        tile_position=(pb, 0))
                    preg[(it, kc)] = (pt, u)
                w = len(pair)
                nc.scalar.activation(pt[:, 0:w * NQ], pss[:, 0:w * NQ],
                                     EXP, scale=0.125)
                budget += rate
                while (fi < len(fillers) and fillers[fi][0] <= tk
                       and fillers[fi][1] <= budget):
                    budget -= fillers[fi][1]
                    fillers[fi][2]()
                    fi += 1
            # endgame: leftover in-stream fillers except the last attnV
            rem = list(fillers[fi:])
            last_av = max(k for k, f in enumerate(rem) if f[3] == 'av')
            for _, _, fn, _ in rem[:last_av]:
                fn()
            # pre-accumulate the t0..t2 partials of three qc=1 output
            # groups while the PE would otherwise wait for the last exp /
            # normalize chain, then run the final attnV and finish up
            parts = [(qc8, oc, emit_wo_part(qc8, oc, T4 - 1))
                     for qc8, oc in wo_groups[8:11]]
            rem[last_av][2]()
            for _, _, fn, _ in rem[last_av + 1:]:
                fn()
            # four more partials in the now-free psS halves (two groups
            # per tile so the 2-buf pool is not wrapped): this work covers
            # the final recip/broadcast/multiply latency
            pstl = [psS.tile([P, 2 * NQ], F32, tag="psS", name="wops2a"),
                    psS.tile([P, 2 * NQ], F32, tag="psS", name="wops2b")]
            for j, (qc8, oc) in enumerate(wo_groups[11:15]):
                acc = pstl[j // 2][:, (j % 2) * NQ:(j % 2 + 1) * NQ]
                for t in range(T4 - 1):
                    nc.tensor.matmul(
                        acc,
                        ct[t][:, qc8 * P:(qc8 + 1) * P],
                        wo_sb[:, t, oc * NQ:(oc + 1) * NQ],
                        start=(t == 0), stop=False)
                parts.append((qc8, oc, acc))
            out_engs.append(nc.scalar)
            for qc8, oc, ps in parts:
                emit_wo_finish(qc8, oc, ps, t0=T4 - 1)
            for qc8, oc in wo_groups[15:]:
                emit_wo_group(qc8, oc)

    nc.compile()
    return nc


def make_in_maps(queries, keys, values, valid_lens, W_q, W_k, W_v, W_o):
    queries = np.asarray(queries, dtype=np.float32)
    keys = np.asarray(keys, dtype=np.float32)
    values = np.asarray(values, dtype=np.float32)
    valid_lens = np.asarray(valid_lens)
    W_q = np.asarray(W_q, dtype=np.float32)
    W_k = np.asarray(W_k, dtype=np.float32)
    W_v = np.asarray(W_v, dtype=np.float32)
    W_o = np.asarray(W_o, dtype=np.float32)

    if MM_DTYPE == 'bf16':
        import ml_dtypes
        f16 = np.dtype(ml_dtypes.bfloat16)
    else:
        f16 = np.float16
    xtq = [np.ascontiguousarray(queries[b].T.astype(f16)) for b in range(B)]
    xtk = [np.ascontiguousarray(keys[b].T.astype(f16)) for b in range(B)]
    xtv = [np.ascontiguousarray(values[b].T.astype(f16)) for b in range(B)]
    wqt = [np.ascontiguousarray(W_q[hg * DL:(hg + 1) * DL, :].T.astype(f16))
           for hg in range(2)]
    wkt = [np.ascontiguousarray(W_k[hg * DL:(hg + 1) * DL, :].T.astype(f16))
           for hg in range(2)]
    wvt = [np.ascontiguousarray(W_v[hg * DL:(hg + 1) * DL, :].T.astype(f16))
           for hg in range(2)]
    wot = [np.ascontiguousarray(W_o[:, hg * DL:(hg + 1) * DL].T.astype(f16))
           for hg in range(2)]

    in_maps = []
    for c in range(N_CORES):
        b, hg = c // 2, c % 2
        L = int(valid_lens[b])
        k_idx = np.arange(S).reshape(KC, P).T  # [P, KC]
        maskd = (k_idx < L).astype(np.float32)
        in_maps.append({
            "xtq": xtq[b], "xtk": xtk[b], "xtv": xtv[b],
            "wq": wqt[hg], "wk": wkt[hg], "wv": wvt[hg], "wo": wot[hg],
            "maskd": np.ascontiguousarray(maskd),
        })
    return in_maps


def gather(results):
    out = np.empty((B, S, D), dtype=np.float32)
    for b in range(B):
        out[b] = (results[2 * b]["out"].astype(np.float32)
                  + results[2 * b + 1]["out"].astype(np.float32))
    return out


def kernel(queries, keys, values, valid_lens, W_q, W_k, W_v, W_o):
    from concourse.bass_utils import run_bass_kernel_spmd

    # key chunks >= ceil(max(valid_lens)/128) are fully masked on every
    # core and contribute exactly zero to numerator and denominator:
    # skip them in the attention loops.
    nkc = max(1, min(KC, -(-int(np.max(np.asarray(valid_lens))) // P)))
    if nkc not in _prog_cache:
        _prog_cache[nkc] = build_program(nkc=nkc)
    nc = _prog_cache[nkc]

    in_maps = make_in_maps(queries, keys, values, valid_lens,
                           W_q, W_k, W_v, W_o)
    res = run_bass_kernel_spmd(nc, in_maps, core_ids=list(range(N_CORES)))
    return gather(res.results)
